# revision 28
# baseline (speedup 1.0000x reference)
"""Trainium2 Bass kernel for nn_Attention_13864154431876.

Dense transformer attention block: QKV projection + RoPE + causal GQA
attention (32 q heads, 8 kv heads, head_dim 128) + output projection.
B=2, S=2048, D=4096, start_pos=0 (cache fully overwritten).

Sharding (8 NeuronCores, tensor parallel by attention heads):
  - each core owns 4 q-heads and 1 kv-head (wq/wk/wv output-dim shards)
  - x is replicated (shipped pre-transposed as x^T so the contraction dim
    lands on partitions)
  - after attention, four AllToAlls (one per 1024-token group)
    redistribute attn^T from head-sharded to token-sharded; each core then
    multiplies its 512-token slab against the full wo and the host
    concatenates the slabs.

All matmuls run in bfloat16 operands with exact fp32 PSUM accumulation.
The per-token-block work is software-pipelined: projections run in two
half-passes (3 PSUM banks each), freeing banks so the previous block's
attention can keep scores 2 steps ahead of the exp->AV chain.
"""
import sys

sys.path.insert(0, "/root/.axon_site/_ro/trn_rl_repo")

import numpy as np
import ml_dtypes

import concourse.bass as bass
import concourse.mybir as mybir
import concourse.tile as tile
from concourse import bacc
from concourse.bass_utils import run_bass_kernel_spmd

F32 = mybir.dt.float32
BF16 = mybir.dt.bfloat16
AF = mybir.ActivationFunctionType
ALU = mybir.AluOpType

N_CORES = 8
B, S, D = 2, 2048, 4096
H, KH, HD = 32, 8, 128
MS = 2048                     # max_seq_len (cache length)
BS = B * S                    # flattened tokens, b-major
HPC = H // N_CORES            # q-heads per core = 4
QF = HPC * HD                 # per-core q-feature width = 512
TB = 512                      # token block
NTB = BS // TB                # 8 token blocks
QBPB = S // TB                # 4 q-blocks per batch element
KC = D // 128                 # 32 contraction chunks
JCB = S // 128                # 16 j-chunks per batch element
SCALE = 1.0 / np.sqrt(HD)
TOKS_PER_CORE = BS // N_CORES  # 512
XPRE = 32                     # xt tiles prefetched before tb0 (full tb)
QTB = TB // 4                 # 128: per-core token slab per a2a group

NPBF = ml_dtypes.bfloat16


def to_bf16(x: np.ndarray) -> np.ndarray:
    return np.ascontiguousarray(np.asarray(x, dtype=np.float32)).astype(NPBF)


def build_attn_nc(mock_collectives=False):
    nc = bacc.Bacc("TRN2", target_bir_lowering=False, debug=False,
                   num_devices=N_CORES)

    # ---- DRAM I/O ----------------------------------------------------
    # xt is host-packed so each 4-kc group tile is one DMA with 4KB
    # contiguous per-partition lines: element (p, tb, kcg, kcs, tok) =
    # x^T[kcg*512 + kcs*128 + p, tb*512 + tok].
    xt_d = nc.dram_tensor("xt", [128, BS * D // 128], BF16,
                          kind="ExternalInput").ap()
    # weights are host-packed partition-major: [128, KC*cols] with
    # element (p, kc*cols + n) = w[kc*128 + p, n], so SBUF loads are
    # large contiguous per-partition runs.
    wq_d = nc.dram_tensor("wq", [128, KC * QF], BF16, kind="ExternalInput").ap()
    wk_d = nc.dram_tensor("wk", [128, KC * HD], BF16, kind="ExternalInput").ap()
    wv_d = nc.dram_tensor("wv", [128, KC * HD], BF16, kind="ExternalInput").ap()
    # wo host-packed: element (p, ob, hc, n) = wo[hc*128 + p, ob*512 + n]
    wo_d = nc.dram_tensor("wo", [128, D * D // 128], BF16,
                          kind="ExternalInput").ap()
    cos_d = nc.dram_tensor("cosT", [HD, S], BF16, kind="ExternalInput").ap()
    sin_d = nc.dram_tensor("sinT", [HD, S], BF16, kind="ExternalInput").ap()
    mask_d = nc.dram_tensor("maskd", [128, 4, TB], BF16, kind="ExternalInput").ap()
    rot_d = nc.dram_tensor("rotm", [HD, HD], BF16, kind="ExternalInput").ap()
    ident_d = nc.dram_tensor("ident", [128, 128], BF16, kind="ExternalInput").ap()
    ones_d = nc.dram_tensor("ones128", [128, 128], BF16, kind="ExternalInput").ap()
    y_d = nc.dram_tensor("y", [TOKS_PER_CORE, D], F32, kind="ExternalOutput").ap()

    # internal DRAM for the four AllToAlls, one per 1024-token group
    # (groups: tbs (0,1), (2,3), (4,5), (6,7)). attn_locG rows are
    # part-major: part p (512 rows = my 512 head-feats) covers 128-token
    # slab p of the group; after A2A, attn_gG rows are global head-feats
    # for MY 128-token slab of the group.
    attn_loc = [nc.dram_tensor(f"attn_loc{g}", [BS, QTB], BF16)
                for g in range(4)]
    attn_g = [nc.dram_tensor(f"attn_g{g}", [D, QTB], BF16)
              for g in range(4)]

    with tile.TileContext(nc) as tc:
        from concourse.tile_rust import add_dep_helper

        # ---- pool stack (bottom-up; closed LIFO) --------------------
        persistB_cm = tc.tile_pool(name="persistB", bufs=1)
        persistB = persistB_cm.__enter__()
        mask_sb = persistB.tile([128, 4, TB], BF16, name="mask_sb")
        rot_sb = persistB.tile([HD, HD], BF16, name="rot_sb")
        ident_sb = persistB.tile([128, 128], BF16, name="ident_sb")
        ones_sb = persistB.tile([128, 128], BF16, name="ones_sb")
        kt_sb = [persistB.tile([HD, S], BF16, name=f"kt{b}_sb") for b in range(B)]
        v_sb = [persistB.tile([128, JCB, HD], BF16, name=f"v{b}_sb") for b in range(B)]

        tier2 = []

        def pool_t2(*a, **kw):
            cm = tc.tile_pool(*a, **kw)
            p = cm.__enter__()
            tier2.append(cm)
            return p

        qtp = pool_t2(name="qtp", bufs=5)
        tmpp = pool_t2(name="tmpp", bufs=2)
        ptp = pool_t2(name="ptp", bufs=4)
        denp = pool_t2(name="denp", bufs=2)
        recbp = pool_t2(name="recbp", bufs=1)
        atp = pool_t2(name="atp", bufs=2)
        asbp = pool_t2(name="asbp", bufs=2)
        # phase-3 pools (persist until the end)
        ap3 = pool_t2(name="attn_sb", bufs=1)
        wop = pool_t2(name="wop", bufs=8)
        ysbp = pool_t2(name="ysb", bufs=2)

        persistA_cm = tc.tile_pool(name="persistA", bufs=1)
        persistA = persistA_cm.__enter__()
        wq_sb = persistA.tile([128, KC, QF], BF16, name="wq_sb")
        wk_sb = persistA.tile([128, KC, HD], BF16, name="wk_sb")
        wv_sb = persistA.tile([128, KC, HD], BF16, name="wv_sb")
        cos_sb = persistA.tile([HD, S], BF16, name="cos_sb")
        sin_sb = persistA.tile([HD, S], BF16, name="sin_sb")

        trans = []

        def pool_tr(*a, **kw):
            cm = tc.tile_pool(*a, **kw)
            p = cm.__enter__()
            trans.append(cm)
            return p

        xtp = pool_tr(name="xtp", bufs=8)

        xtr = xt_d.rearrange("p (tb kcg kcs tok) -> p tb kcg kcs tok",
                             tb=NTB, kcg=KC // 4, kcs=4)

        def xt_dma(tb, kcg):
            t = xtp.tile([128, 4, TB], BF16, name="xt_t", tag="xt")
            nc.sync.dma_start(t[:], xtr[:, tb, kcg, :, :])
            return t
        qrawp = pool_tr(name="qrawp", bufs=2)
        vtrawp = pool_tr(name="vtrawp", bufs=2)
        # PSUM: 3 projection accumulators (half-pass) + 3 score banks
        # (lag-2 score/AV pipeline) + 2 attention accumulators = 8.
        pp_cm = tc.tile_pool(name="pp", bufs=3, space="PSUM")
        pp = pp_cm.__enter__()
        ps_cm = tc.tile_pool(name="ps", bufs=3, space="PSUM")
        ps = ps_cm.__enter__()
        pa_cm = tc.tile_pool(name="pa", bufs=2, space="PSUM")
        pa = pa_cm.__enter__()

        # ---- startup DMA order: critical path first -----------------
        # weights arrive in 4 big chunked DMAs each (8 kc per chunk =
        # 8KB contiguous per partition), interleaved with tb0's xt tiles.
        pre_xt = []
        wqr = wq_d.rearrange("p (kc n) -> p kc n", n=QF)
        wkr = wk_d.rearrange("p (kc n) -> p kc n", n=HD)
        wvr = wv_d.rearrange("p (kc n) -> p kc n", n=HD)
        for blk in range(4):
            sl = slice(blk * 8, (blk + 1) * 8)
            if blk == 0:
                nc.sync.dma_start(wq_sb[:, 0:2, :], wqr[:, 0:2, :])
                nc.sync.dma_start(wq_sb[:, 2:8, :], wqr[:, 2:8, :])
            else:
                nc.sync.dma_start(wq_sb[:, sl, :], wqr[:, sl, :])
            nc.sync.dma_start(wk_sb[:, sl, :], wkr[:, sl, :])
            nc.sync.dma_start(wv_sb[:, sl, :], wvr[:, sl, :])
            for kcg in range(blk * 2, (blk + 1) * 2):
                if kcg == 0:
                    # first group: 4 separate per-kc DMAs so the very
                    # first matmul starts ~3x sooner
                    t = xtp.tile([128, 4, TB], BF16, name="xt_t", tag="xt")
                    for kcs in range(4):
                        nc.sync.dma_start(t[:, kcs, :],
                                          xtr[:, 0, 0, kcs, :])
                    pre_xt.append(t)
                else:
                    pre_xt.append(xt_dma(0, kcg))
            if blk == 0:
                nc.sync.dma_start(rot_sb[:], rot_d[:])
                nc.sync.dma_start(ident_sb[:], ident_d[:])
                nc.sync.dma_start(ones_sb[:], ones_d[:])
            if blk == 1:
                nc.sync.dma_start(cos_sb[:], cos_d[:])
                nc.sync.dma_start(sin_sb[:], sin_d[:])
                nc.sync.dma_start(mask_sb[:], mask_d[:])

        def emit_attention(tb, ps_pool, pa_pool, qt_tiles, lag=2):
            """Generator: attention for token block tb, software-pipelined
            so each score matmul runs `lag` steps ahead of its AV matmul
            (covering the exp latency on the Scalar engine)."""
            b, qb = tb // QBPB, tb % QBPB
            njc = (qb + 1) * 4
            grp = tb // 2
            for h in range(HPC):
                denacc = denp.tile([128, TB], F32, name="denacc", tag="den")
                denf = denp.tile([128, TB], BF16, name="denf", tag="den")
                aps = pa_pool.tile([128, TB], F32, name="aps", tag="aps")
                pts = {}

                def emit_score(jc):
                    # diagonal chunks: columns < r*128 are fully masked;
                    # skip them in the score MM, exp, mask and den ops.
                    r = jc - qb * 4
                    c0 = r * 128 if r > 0 else 0
                    sps = ps_pool.tile([128, TB], F32, name="sps", tag="sps")
                    nc.tensor.matmul(
                        sps[:, c0:], kt_sb[b][:, jc * 128:(jc + 1) * 128],
                        qt_tiles[h][:, c0:], start=True, stop=True,
                        skip_group_check=True)
                    pt = ptp.tile([128, TB], BF16, name="pt", tag="pt")
                    if r >= 0:
                        praw = tmpp.tile([128, TB], BF16, name="praw",
                                         tag="tmp")
                        nc.scalar.activation(praw[:, c0:], sps[:, c0:],
                                             AF.Exp)
                        nc.vector.tensor_tensor(pt[:, c0:], praw[:, c0:],
                                                mask_sb[:, r, c0:], ALU.mult)
                    else:
                        nc.scalar.activation(pt[:], sps[:], AF.Exp)
                    if jc == 0:
                        nc.vector.tensor_copy(denacc[:], pt[:])
                    elif jc == njc - 1:
                        nc.vector.tensor_tensor(denf[:, c0:], denacc[:, c0:],
                                                pt[:, c0:], ALU.add)
                        nc.vector.tensor_copy(denf[:, :c0], denacc[:, :c0])
                    else:
                        nc.vector.tensor_tensor(denacc[:, c0:],
                                                denacc[:, c0:],
                                                pt[:, c0:], ALU.add)
                    pts[jc] = (pt, c0)

                for j in range(njc + lag):
                    if j < njc:
                        emit_score(j)
                        yield
                    if j - lag >= 0:
                        jc = j - lag
                        pt, c0 = pts.pop(jc)
                        nc.tensor.matmul(
                            aps[:, c0:], v_sb[b][:, jc, :], pt[:, c0:],
                            start=(jc == 0), stop=(jc == njc - 1),
                            skip_group_check=True)
                        yield
                # colsum+broadcast in one ones-matmul; fast reciprocal
                asb = asbp.tile([128, TB], BF16, name="asb", tag="asb")
                nc.vector.tensor_copy(asb[:], aps[:])
                denb = ps_pool.tile([128, TB], F32, name="denb", tag="sps")
                nc.tensor.matmul(denb[:], ones_sb[:], denf[:],
                                 start=True, stop=True, skip_group_check=True)
                recipb = recbp.tile([128, TB], F32, name="recipb", tag="recb")
                nc.vector.reciprocal_approx_fast(recipb[:], denb[:])
                attn_t = atp.tile([128, TB], BF16, name="attn_t", tag="attn_t")
                nc.vector.tensor_tensor(attn_t[:], asb[:], recipb[:], ALU.mult)
                lb = tb % 2
                for qt4 in range(4):
                    nc.sync.dma_start(
                        attn_loc[grp].ap()[
                            (4 * lb + qt4) * 512 + h * 128:
                            (4 * lb + qt4) * 512 + (h + 1) * 128, :],
                        attn_t[:, qt4 * QTB:(qt4 + 1) * QTB])
                yield

        def drive(gen, n):
            if gen is None:
                return None
            for _ in range(n):
                try:
                    next(gen)
                except StopIteration:
                    return None
            return gen

        def emit_a2a(g):
            if mock_collectives:
                return nc.sync.dma_start(attn_g[g].ap()[:], attn_loc[g].ap()[:])
            # f32-bitcast views: the collective path is ~4x slower on
            # 2-byte dtypes, and AllToAll(bypass) only moves bytes.
            return nc.gpsimd.collective_compute(
                "AllToAll", ALU.bypass,
                replica_groups=[list(range(N_CORES))],
                ins=[attn_loc[g].ap().bitcast(F32).opt()],
                outs=[attn_g[g].ap().bitcast(F32).opt()],
            )

        wo_r = wo_d.rearrange("p (ob hc n) -> p ob hc n", ob=8, hc=KC)

        def load_wo_block(ob):
            tiles = []
            for g in range(8):
                wt = wop.tile([128, 4, TB], BF16, name="wo_t", tag="wo")
                nc.sync.dma_start(
                    wt[:], wo_r[:, ob, g * 4:(g + 1) * 4, :])
                tiles.append(wt)
            return tiles

        attn_sb = [None] * 4

        def load_attn_sb(g, cc, half=None):
            # per-hc loads: each is a contiguous 128-row block of attn_g,
            # so every partition reads one contiguous 256B run. `half`
            # splits the 1MB burst across two emission points.
            if half in (None, 0):
                attn_sb[g] = ap3.tile([128, KC, 128], BF16,
                                      name=f"attn_sb{g}")
            t = attn_sb[g]
            hcs = range(KC) if half is None else \
                range(half * KC // 2, (half + 1) * KC // 2)
            for hc in hcs:
                dma = nc.sync.dma_start(
                    t[:, hc, :],
                    attn_g[g].ap()[hc * 128:(hc + 1) * 128, :])
                if cc is not None:
                    add_dep_helper(dma.ins, cc,
                                   reason="attn_sb load after AllToAll")

        wo_g0 = None
        cc_ins = {}
        prev_gen = None
        prev_steps = 0
        for tb in range(NTB):
            b, qb = tb // QBPB, tb % QBPB
            s0 = qb * TB
            resv = 20 if prev_steps > 20 else 0
            per_kc = max(1, -(-(prev_steps - resv) // (2 * KC)))  # ceil
            # ---- projections for tb in two half-passes, interleaved
            # with attention(tb-1); rope for the first half between them.
            # tb0 has no attention to interleave, so it runs a single
            # fused pass (6 MMs per xt tile — half the DMA supply rate),
            # borrowing the idle ps/pa banks for the extra accumulators.
            qps01 = [pp.tile([128, TB], F32, name=f"qps{h}", tag="proj")
                     for h in range(2)]
            kps = pp.tile([128, TB], F32, name="kps", tag="proj")
            if tb == 0:
                qps23 = [ps.tile([128, TB], F32, name=f"qps{h+2}", tag="sps")
                         for h in range(2)]
                vtps = pa.tile([128, TB], F32, name="vtps", tag="aps")
            for kc in range(KC):
                xt_t = pre_xt[kc // 4][:, kc % 4, :]
                for h in range(2):
                    nc.tensor.matmul(
                        qps01[h][:], wq_sb[:, kc, h * 128:(h + 1) * 128],
                        xt_t, start=(kc == 0), stop=(kc == KC - 1),
                        skip_group_check=True)
                nc.tensor.matmul(kps[:], wk_sb[:, kc, :], xt_t,
                                 start=(kc == 0), stop=(kc == KC - 1),
                                 skip_group_check=True)
                if tb == 0:
                    for h in range(2):
                        nc.tensor.matmul(
                            qps23[h][:],
                            wq_sb[:, kc, (h + 2) * 128:(h + 3) * 128],
                            xt_t, start=(kc == 0), stop=(kc == KC - 1),
                            skip_group_check=True)
                    nc.tensor.matmul(vtps[:], wv_sb[:, kc, :], xt_t,
                                     start=(kc == 0), stop=(kc == KC - 1),
                                     skip_group_check=True)
                    if kc % 4 == 3:
                        pre_xt[kc // 4] = xt_dma(1, kc // 4)
                prev_gen = drive(prev_gen, per_kc)

            # rope for q0, q1 and K (drains first-pass PSUM banks)
            qt_tiles = []

            def rope_unit(src_ps, dst_ap):
                """dst = src*cos + rotate_half(src)*sin (PSUM f32 -> bf16)"""
                raw = qrawp.tile([128, TB], BF16, name="qraw", tag="qraw")
                nc.scalar.activation(raw[:], src_ps[:], AF.Copy)
                rotps = ps.tile([128, TB], F32, name="rotps", tag="sps")
                nc.tensor.matmul(rotps[:], rot_sb[:], raw[:],
                                 start=True, stop=True, skip_group_check=True)
                tcos = tmpp.tile([128, TB], F32, name="tcos", tag="tmp")
                nc.vector.tensor_tensor(tcos[:], raw[:],
                                        cos_sb[:, s0:s0 + TB], ALU.mult)
                tsin = tmpp.tile([128, TB], F32, name="tsin", tag="tmp")
                nc.vector.tensor_tensor(tsin[:], rotps[:],
                                        sin_sb[:, s0:s0 + TB], ALU.mult)
                nc.vector.tensor_tensor(dst_ap, tcos[:], tsin[:], ALU.add)

            for h in range(2):
                qt = qtp.tile([128, TB], BF16, name="qt", tag="qt")
                rope_unit(qps01[h], qt[:])
                qt_tiles.append(qt)
                prev_gen = drive(prev_gen, 2)
            rope_unit(kps, kt_sb[b][:, s0:s0 + TB])
            prev_gen = drive(prev_gen, 2)

            # ---- second half-pass: q2, q3, V -----------------------
            if tb > 0:
                qps23 = [pp.tile([128, TB], F32, name=f"qps{h+2}",
                                 tag="proj") for h in range(2)]
                vtps = pp.tile([128, TB], F32, name="vtps", tag="proj")
                for kc in range(KC):
                    xt_t = pre_xt[kc // 4][:, kc % 4, :]
                    for h in range(2):
                        nc.tensor.matmul(
                            qps23[h][:],
                            wq_sb[:, kc, (h + 2) * 128:(h + 3) * 128],
                            xt_t, start=(kc == 0), stop=(kc == KC - 1),
                            skip_group_check=True)
                    nc.tensor.matmul(vtps[:], wv_sb[:, kc, :], xt_t,
                                     start=(kc == 0), stop=(kc == KC - 1),
                                     skip_group_check=True)
                    # one-tb-ahead xt prefetch (after last read of group)
                    if tb < NTB - 1 and kc % 4 == 3:
                        pre_xt[kc // 4] = xt_dma(tb + 1, kc // 4)
                    prev_gen = drive(prev_gen, per_kc)

            for h in range(2):
                qt = qtp.tile([128, TB], BF16, name="qt", tag="qt")
                rope_unit(qps23[h], qt[:])
                qt_tiles.append(qt)
                prev_gen = drive(prev_gen, 2)
            # V: drain V^T then transpose 4x [128,128]
            vtraw = vtrawp.tile([128, TB], BF16, name="vtraw", tag="vtraw")
            nc.scalar.activation(vtraw[:], vtps[:], AF.Copy)
            vtr = pp.tile([128, TB], BF16, name="vtr", tag="proj")
            for t4 in range(4):
                nc.tensor.transpose(vtr[:, t4 * 128:(t4 + 1) * 128],
                                    vtraw[:, t4 * 128:(t4 + 1) * 128],
                                    ident_sb[:])
            nc.vector.tensor_copy(
                v_sb[b].rearrange("p jc d -> p (jc d)")[:, s0:s0 + TB],
                vtr[:])
            prev_gen = drive(prev_gen, 10 ** 9)  # flush any leftovers
            # fire the A2A for the group completed by attention(tb-1);
            # the burst overlaps the next tb's dense projection stream.
            if tb in (2, 4, 6):
                g = tb // 2 - 1
                cc = emit_a2a(g)
                cc_ins[g] = cc.ins
            if tb < NTB - 1:
                prev_gen = emit_attention(tb, ps, pa, qt_tiles)
                prev_steps = HPC * (2 * ((qb + 1) * 4) + 2)
                if tb == 3:
                    load_attn_sb(0, cc_ins[0], half=0)
                if tb == 4:
                    load_attn_sb(0, cc_ins[0], half=1)
                    wo_g0 = load_wo_block(0)
                if tb == 5:
                    load_attn_sb(1, cc_ins[1], half=0)
                if tb == 6:
                    load_attn_sb(1, cc_ins[1], half=1)
            else:
                load_attn_sb(2, cc_ins[2], half=0)
                tail_qt = qt_tiles

        # ---- free projection-only pools -----------------------------
        pa_cm.__exit__(None, None, None)
        ps_cm.__exit__(None, None, None)
        pp_cm.__exit__(None, None, None)
        for cm in reversed(trans):
            cm.__exit__(None, None, None)
        persistA_cm.__exit__(None, None, None)

        # ---- attention tail (tb=7) with generous psum buffering -----
        ps2_cm = tc.tile_pool(name="ps2", bufs=5, space="PSUM")
        ps2 = ps2_cm.__enter__()
        pa2_cm = tc.tile_pool(name="pa2", bufs=2, space="PSUM")
        pa2 = pa2_cm.__enter__()
        tail_gen = emit_attention(NTB - 1, ps2, pa2, tail_qt, lag=3)
        drive(tail_gen, 40)
        load_attn_sb(2, cc_ins[2], half=1)
        drive(tail_gen, 10 ** 9)
        cc3 = emit_a2a(3)
        load_attn_sb(3, cc3.ins)
        pa2_cm.__exit__(None, None, None)
        ps2_cm.__exit__(None, None, None)

        # ---- phase 3: y = attn_rows @ wo ----------------------------
        py_cm = tc.tile_pool(name="py", bufs=4, space="PSUM")
        pyp = py_cm.__enter__()
        for ob in range(8):
            wo_g = wo_g0 if ob == 0 else load_wo_block(ob)
            for tc4 in range(4):
                yps = pyp.tile([128, TB], F32, name="yps", tag="yps")
                for hc in range(KC):
                    nc.tensor.matmul(
                        yps[:], attn_sb[tc4][:, hc, :],
                        wo_g[hc // 4][:, hc % 4, :],
                        start=(hc == 0), stop=(hc == KC - 1),
                        skip_group_check=True)
                y_sb = ysbp.tile([128, TB], F32, name="y_sb", tag="y")
                nc.vector.tensor_copy(y_sb[:], yps[:])
                nc.sync.dma_start(
                    y_d[tc4 * 128:(tc4 + 1) * 128,
                        ob * TB:(ob + 1) * TB], y_sb[:])
        py_cm.__exit__(None, None, None)
        for cm in reversed(tier2):
            cm.__exit__(None, None, None)
        persistB_cm.__exit__(None, None, None)

    nc.compile()
    return nc



_NC_CACHE = None


def _get_nc():
    global _NC_CACHE
    if _NC_CACHE is None:
        _NC_CACHE = build_attn_nc()
    return _NC_CACHE


def _host_reference(x, wq, wk, wv, wo, sincos, start_pos, causal_mask):
    """Numpy fallback (only used if the mask is not causal-tril)."""
    xq = (x @ wq).reshape(B, S, H, HD)
    xk = (x @ wk).reshape(B, S, KH, HD)
    xv = (x @ wv).reshape(B, S, KH, HD)
    sp = min(max(int(start_pos), 0), MS - S)
    sc = sincos[sp:sp + S]
    sin, cos = sc[:, :HD], sc[:, HD:]
    sin = sin[None, :, None, :]
    cos = cos[None, :, None, :]

    def rot(u):
        return np.concatenate([-u[..., HD // 2:], u[..., :HD // 2]], axis=-1)

    xq = xq * cos + rot(xq) * sin
    xk = xk * cos + rot(xk) * sin
    mask = np.broadcast_to(causal_mask[:, sp:sp + S, :MS], (B, S, MS))
    out = np.zeros((B, S, H, HD), dtype=np.float32)
    nrep = H // KH
    for b in range(B):
        for h in range(H):
            q = xq[b, :, h]
            k = xk[b, :, h // nrep]
            v = xv[b, :, h // nrep]
            s = (q @ k.T) * SCALE
            s = np.where(mask[b], s, -np.inf)
            s = s - s.max(axis=-1, keepdims=True)
            p = np.exp(s)
            p /= p.sum(axis=-1, keepdims=True)
            out[b, :, h] = p @ v
    return out.reshape(B, S, H * HD) @ wo


def kernel(x, wq, wk, wv, wo, cache_k, cache_v, sincos, causal_mask,
           start_pos):
    x = np.asarray(x, dtype=np.float32)
    wq = np.asarray(wq, dtype=np.float32)
    wk = np.asarray(wk, dtype=np.float32)
    wv = np.asarray(wv, dtype=np.float32)
    wo = np.asarray(wo, dtype=np.float32)
    sincos = np.asarray(sincos, dtype=np.float32)
    cm = np.asarray(causal_mask)
    sp = min(max(int(start_pos), 0), MS - S)

    tril = np.tril(np.ones((S, MS), dtype=bool))
    if not np.array_equal(cm[0, sp:sp + S, :], tril[:, :MS]):
        return _host_reference(x, wq, wk, wv, wo, sincos, start_pos,
                               cm).astype(np.float32)

    # host prep
    sc = sincos[sp:sp + S]
    sinT = np.ascontiguousarray(sc[:, :HD].T)       # [HD, S]
    cosT = np.ascontiguousarray(sc[:, HD:].T)       # [HD, S]
    xtT = x.reshape(BS, D).T  # [D, BS]
    # pack to (p, tb, kcg, kcs, tok)
    xt = to_bf16(np.ascontiguousarray(
        xtT.reshape(KC // 4, 4, 128, NTB, TB)
           .transpose(2, 3, 0, 1, 4).reshape(128, -1)))
    wqs = wq * np.float32(SCALE)
    wo_b = to_bf16(np.ascontiguousarray(
        wo.reshape(KC, 128, 8, TB).transpose(1, 2, 0, 3).reshape(128, -1)))

    maskd = np.zeros((128, 4, TB), dtype=np.float32)
    j = np.arange(128)[:, None, None]
    r = np.arange(4)[None, :, None]
    q = np.arange(TB)[None, None, :]
    maskd[(r * 128 + j) <= q] = 1.0

    rotm = np.zeros((HD, HD), dtype=np.float32)
    hh = HD // 2
    rotm[np.arange(hh) + hh, np.arange(hh)] = -1.0
    rotm[np.arange(hh), np.arange(hh) + hh] = 1.0

    ident = np.eye(128, dtype=np.float32)
    ones128 = np.ones((128, 128), dtype=np.float32)

    def pack_w(w):
        # [D, cols] -> partition-major [128, KC*cols]
        cols = w.shape[1]
        return to_bf16(np.ascontiguousarray(
            w.reshape(KC, 128, cols).transpose(1, 0, 2).reshape(128, -1)))

    in_maps = []
    for c in range(N_CORES):
        in_maps.append({
            "xt": xt,
            "wq": pack_w(wqs[:, c * QF:(c + 1) * QF]),
            "wk": pack_w(wk[:, c * HD:(c + 1) * HD]),
            "wv": pack_w(wv[:, c * HD:(c + 1) * HD]),
            "wo": wo_b,
            "cosT": to_bf16(cosT), "sinT": to_bf16(sinT),
            "maskd": to_bf16(maskd), "rotm": to_bf16(rotm),
            "ident": to_bf16(ident),
            "ones128": to_bf16(ones128),
        })

    global _LAST_IN_MAPS
    _LAST_IN_MAPS = in_maps
    nc = _get_nc()
    res = run_bass_kernel_spmd(nc, in_maps, list(range(N_CORES)))
    # per-core y rows: 4 slabs of 128 tokens, one per a2a token group g
    # (global tokens g*1024 + c*128 .. +128, b-major flat order).
    y = np.empty((BS, D), dtype=np.float32)
    for c in range(N_CORES):
        yc = res.results[c]["y"]
        for g in range(4):
            y[g * 1024 + c * 128:g * 1024 + (c + 1) * 128] = \
                yc[g * 128:(g + 1) * 128]
    return y.reshape(B, S, D)


# revision 29
# speedup vs baseline: 1.0046x; 1.0046x over previous
"""Trainium2 Bass kernel for nn_Attention_13864154431876.

Dense transformer attention block: QKV projection + RoPE + causal GQA
attention (32 q heads, 8 kv heads, head_dim 128) + output projection.
B=2, S=2048, D=4096, start_pos=0 (cache fully overwritten).

Sharding (8 NeuronCores, tensor parallel by attention heads):
  - each core owns 4 q-heads and 1 kv-head (wq/wk/wv output-dim shards)
  - x is replicated (shipped pre-transposed as x^T so the contraction dim
    lands on partitions)
  - after attention, four AllToAlls (one per 1024-token group)
    redistribute attn^T from head-sharded to token-sharded; each core then
    multiplies its 512-token slab against the full wo and the host
    concatenates the slabs.

All matmuls run in bfloat16 operands with exact fp32 PSUM accumulation.
The per-token-block work is software-pipelined: projections run in two
half-passes (3 PSUM banks each), freeing banks so the previous block's
attention can keep scores 2 steps ahead of the exp->AV chain.
"""
import sys

sys.path.insert(0, "/root/.axon_site/_ro/trn_rl_repo")

import numpy as np
import ml_dtypes

import concourse.bass as bass
import concourse.mybir as mybir
import concourse.tile as tile
from concourse import bacc
from concourse.bass_utils import run_bass_kernel_spmd

F32 = mybir.dt.float32
BF16 = mybir.dt.bfloat16
AF = mybir.ActivationFunctionType
ALU = mybir.AluOpType

N_CORES = 8
B, S, D = 2, 2048, 4096
H, KH, HD = 32, 8, 128
MS = 2048                     # max_seq_len (cache length)
BS = B * S                    # flattened tokens, b-major
HPC = H // N_CORES            # q-heads per core = 4
QF = HPC * HD                 # per-core q-feature width = 512
TB = 512                      # token block
NTB = BS // TB                # 8 token blocks
QBPB = S // TB                # 4 q-blocks per batch element
KC = D // 128                 # 32 contraction chunks
JCB = S // 128                # 16 j-chunks per batch element
SCALE = 1.0 / np.sqrt(HD)
TOKS_PER_CORE = BS // N_CORES  # 512
XPRE = 32                     # xt tiles prefetched before tb0 (full tb)
QTB = TB // 4                 # 128: per-core token slab per a2a group

NPBF = ml_dtypes.bfloat16


def to_bf16(x: np.ndarray) -> np.ndarray:
    return np.ascontiguousarray(np.asarray(x, dtype=np.float32)).astype(NPBF)


def build_attn_nc(mock_collectives=False):
    nc = bacc.Bacc("TRN2", target_bir_lowering=False, debug=False,
                   num_devices=N_CORES)

    # ---- DRAM I/O ----------------------------------------------------
    # xt is host-packed so each 4-kc group tile is one DMA with 4KB
    # contiguous per-partition lines: element (p, tb, kcg, kcs, tok) =
    # x^T[kcg*512 + kcs*128 + p, tb*512 + tok].
    xt_d = nc.dram_tensor("xt", [128, BS * D // 128], BF16,
                          kind="ExternalInput").ap()
    # weights are host-packed partition-major: [128, KC*cols] with
    # element (p, kc*cols + n) = w[kc*128 + p, n], so SBUF loads are
    # large contiguous per-partition runs.
    wq_d = nc.dram_tensor("wq", [128, KC * QF], BF16, kind="ExternalInput").ap()
    wk_d = nc.dram_tensor("wk", [128, KC * HD], BF16, kind="ExternalInput").ap()
    wv_d = nc.dram_tensor("wv", [128, KC * HD], BF16, kind="ExternalInput").ap()
    # wo host-packed: element (p, ob, hc, n) = wo[hc*128 + p, ob*512 + n]
    wo_d = nc.dram_tensor("wo", [128, D * D // 128], BF16,
                          kind="ExternalInput").ap()
    cos_d = nc.dram_tensor("cosT", [HD, S], BF16, kind="ExternalInput").ap()
    sin_d = nc.dram_tensor("sinT", [HD, S], BF16, kind="ExternalInput").ap()
    mask_d = nc.dram_tensor("maskd", [128, 4, TB], BF16, kind="ExternalInput").ap()
    rot_d = nc.dram_tensor("rotm", [HD, HD], BF16, kind="ExternalInput").ap()
    ident_d = nc.dram_tensor("ident", [128, 128], BF16, kind="ExternalInput").ap()
    ones_d = nc.dram_tensor("ones128", [128, 128], BF16, kind="ExternalInput").ap()
    y_d = nc.dram_tensor("y", [TOKS_PER_CORE, D], F32, kind="ExternalOutput").ap()

    # internal DRAM for the four AllToAlls, one per 1024-token group
    # (groups: tbs (0,1), (2,3), (4,5), (6,7)). attn_locG rows are
    # part-major: part p (512 rows = my 512 head-feats) covers 128-token
    # slab p of the group; after A2A, attn_gG rows are global head-feats
    # for MY 128-token slab of the group.
    attn_loc = [nc.dram_tensor(f"attn_loc{g}", [BS, QTB], BF16)
                for g in range(4)]
    attn_g = [nc.dram_tensor(f"attn_g{g}", [D, QTB], BF16)
              for g in range(4)]

    with tile.TileContext(nc) as tc:
        from concourse.tile_rust import add_dep_helper

        # ---- pool stack (bottom-up; closed LIFO) --------------------
        persistB_cm = tc.tile_pool(name="persistB", bufs=1)
        persistB = persistB_cm.__enter__()
        mask_sb = persistB.tile([128, 4, TB], BF16, name="mask_sb")
        rot_sb = persistB.tile([HD, HD], BF16, name="rot_sb")
        ident_sb = persistB.tile([128, 128], BF16, name="ident_sb")
        ones_sb = persistB.tile([128, 128], BF16, name="ones_sb")
        kt_sb = [persistB.tile([HD, S], BF16, name=f"kt{b}_sb") for b in range(B)]
        v_sb = [persistB.tile([128, JCB, HD], BF16, name=f"v{b}_sb") for b in range(B)]

        tier2 = []

        def pool_t2(*a, **kw):
            cm = tc.tile_pool(*a, **kw)
            p = cm.__enter__()
            tier2.append(cm)
            return p

        qtp = pool_t2(name="qtp", bufs=5)
        tmpp = pool_t2(name="tmpp", bufs=2)
        ptp = pool_t2(name="ptp", bufs=4)
        denp = pool_t2(name="denp", bufs=2)
        recbp = pool_t2(name="recbp", bufs=1)
        atp = pool_t2(name="atp", bufs=2)
        asbp = pool_t2(name="asbp", bufs=2)
        # phase-3 pools (persist until the end)
        ap3 = pool_t2(name="attn_sb", bufs=1)
        wop = pool_t2(name="wop", bufs=8)
        ysbp = pool_t2(name="ysb", bufs=2)

        persistA_cm = tc.tile_pool(name="persistA", bufs=1)
        persistA = persistA_cm.__enter__()
        wq_sb = persistA.tile([128, KC, QF], BF16, name="wq_sb")
        wk_sb = persistA.tile([128, KC, HD], BF16, name="wk_sb")
        wv_sb = persistA.tile([128, KC, HD], BF16, name="wv_sb")
        cos_sb = persistA.tile([HD, S], BF16, name="cos_sb")
        sin_sb = persistA.tile([HD, S], BF16, name="sin_sb")

        trans = []

        def pool_tr(*a, **kw):
            cm = tc.tile_pool(*a, **kw)
            p = cm.__enter__()
            trans.append(cm)
            return p

        xtp = pool_tr(name="xtp", bufs=8)

        xtr = xt_d.rearrange("p (tb kcg kcs tok) -> p tb kcg kcs tok",
                             tb=NTB, kcg=KC // 4, kcs=4)

        def xt_dma(tb, kcg):
            t = xtp.tile([128, 4, TB], BF16, name="xt_t", tag="xt")
            nc.sync.dma_start(t[:], xtr[:, tb, kcg, :, :])
            return t
        qrawp = pool_tr(name="qrawp", bufs=2)
        vtrawp = pool_tr(name="vtrawp", bufs=2)
        # PSUM: 3 projection accumulators (half-pass) + 3 score banks
        # (lag-2 score/AV pipeline) + 2 attention accumulators = 8.
        pp_cm = tc.tile_pool(name="pp", bufs=3, space="PSUM")
        pp = pp_cm.__enter__()
        ps_cm = tc.tile_pool(name="ps", bufs=3, space="PSUM")
        ps = ps_cm.__enter__()
        pa_cm = tc.tile_pool(name="pa", bufs=2, space="PSUM")
        pa = pa_cm.__enter__()

        # ---- startup DMA order: critical path first -----------------
        # weights arrive in 4 big chunked DMAs each (8 kc per chunk =
        # 8KB contiguous per partition), interleaved with tb0's xt tiles.
        pre_xt = []
        wqr = wq_d.rearrange("p (kc n) -> p kc n", n=QF)
        wkr = wk_d.rearrange("p (kc n) -> p kc n", n=HD)
        wvr = wv_d.rearrange("p (kc n) -> p kc n", n=HD)
        for blk in range(4):
            sl = slice(blk * 8, (blk + 1) * 8)
            if blk == 0:
                nc.sync.dma_start(wq_sb[:, 0:2, :], wqr[:, 0:2, :])
                nc.sync.dma_start(wq_sb[:, 2:8, :], wqr[:, 2:8, :])
            else:
                nc.sync.dma_start(wq_sb[:, sl, :], wqr[:, sl, :])
            nc.sync.dma_start(wk_sb[:, sl, :], wkr[:, sl, :])
            nc.sync.dma_start(wv_sb[:, sl, :], wvr[:, sl, :])
            for kcg in range(blk * 2, (blk + 1) * 2):
                if kcg == 0:
                    # first group: 4 separate per-kc DMAs so the very
                    # first matmul starts ~3x sooner
                    t = xtp.tile([128, 4, TB], BF16, name="xt_t", tag="xt")
                    for kcs in range(4):
                        nc.sync.dma_start(t[:, kcs, :],
                                          xtr[:, 0, 0, kcs, :])
                    pre_xt.append(t)
                else:
                    pre_xt.append(xt_dma(0, kcg))
            if blk == 0:
                nc.sync.dma_start(rot_sb[:], rot_d[:])
                nc.sync.dma_start(ident_sb[:], ident_d[:])
                nc.sync.dma_start(ones_sb[:], ones_d[:])
            if blk == 1:
                nc.sync.dma_start(cos_sb[:], cos_d[:])
                nc.sync.dma_start(sin_sb[:], sin_d[:])
                nc.sync.dma_start(mask_sb[:], mask_d[:])

        def emit_attention(tb, ps_pool, pa_pool, qt_tiles, lag=2):
            """Generator: attention for token block tb, software-pipelined
            so each score matmul runs `lag` steps ahead of its AV matmul
            (covering the exp latency on the Scalar engine)."""
            b, qb = tb // QBPB, tb % QBPB
            njc = (qb + 1) * 4
            grp = tb // 2
            for h in range(HPC):
                denacc = denp.tile([128, TB], F32, name="denacc", tag="den")
                denf = denp.tile([128, TB], BF16, name="denf", tag="den")
                aps = pa_pool.tile([128, TB], F32, name="aps", tag="aps")
                pts = {}

                def emit_score(jc):
                    # diagonal chunks: columns < r*128 are fully masked;
                    # skip them in the score MM, exp, mask and den ops.
                    r = jc - qb * 4
                    c0 = r * 128 if r > 0 else 0
                    sps = ps_pool.tile([128, TB], F32, name="sps", tag="sps")
                    nc.tensor.matmul(
                        sps[:, c0:], kt_sb[b][:, jc * 128:(jc + 1) * 128],
                        qt_tiles[h][:, c0:], start=True, stop=True,
                        skip_group_check=True)
                    pt = ptp.tile([128, TB], BF16, name="pt", tag="pt")
                    if r >= 0:
                        praw = tmpp.tile([128, TB], BF16, name="praw",
                                         tag="tmp")
                        nc.scalar.activation(praw[:, c0:], sps[:, c0:],
                                             AF.Exp)
                        nc.vector.tensor_tensor(pt[:, c0:], praw[:, c0:],
                                                mask_sb[:, r, c0:], ALU.mult)
                    else:
                        nc.scalar.activation(pt[:], sps[:], AF.Exp)
                    if jc == 0:
                        nc.vector.tensor_copy(denacc[:], pt[:])
                    elif jc == njc - 1:
                        nc.vector.tensor_tensor(denf[:, c0:], denacc[:, c0:],
                                                pt[:, c0:], ALU.add)
                        nc.vector.tensor_copy(denf[:, :c0], denacc[:, :c0])
                    else:
                        nc.vector.tensor_tensor(denacc[:, c0:],
                                                denacc[:, c0:],
                                                pt[:, c0:], ALU.add)
                    pts[jc] = (pt, c0)

                for j in range(njc + lag):
                    if j < njc:
                        emit_score(j)
                        yield
                    if j - lag >= 0:
                        jc = j - lag
                        pt, c0 = pts.pop(jc)
                        nc.tensor.matmul(
                            aps[:, c0:], v_sb[b][:, jc, :], pt[:, c0:],
                            start=(jc == 0), stop=(jc == njc - 1),
                            skip_group_check=True)
                        yield
                # colsum+broadcast in one ones-matmul; fast reciprocal
                asb = asbp.tile([128, TB], BF16, name="asb", tag="asb")
                nc.vector.tensor_copy(asb[:], aps[:])
                denb = ps_pool.tile([128, TB], F32, name="denb", tag="sps")
                nc.tensor.matmul(denb[:], ones_sb[:], denf[:],
                                 start=True, stop=True, skip_group_check=True)
                recipb = recbp.tile([128, TB], F32, name="recipb", tag="recb")
                nc.vector.reciprocal_approx_fast(recipb[:], denb[:])
                attn_t = atp.tile([128, TB], BF16, name="attn_t", tag="attn_t")
                nc.vector.tensor_tensor(attn_t[:], asb[:], recipb[:], ALU.mult)
                lb = tb % 2
                for qt4 in range(4):
                    nc.sync.dma_start(
                        attn_loc[grp].ap()[
                            (4 * lb + qt4) * 512 + h * 128:
                            (4 * lb + qt4) * 512 + (h + 1) * 128, :],
                        attn_t[:, qt4 * QTB:(qt4 + 1) * QTB])
                yield

        def drive(gen, n):
            if gen is None:
                return None
            for _ in range(n):
                try:
                    next(gen)
                except StopIteration:
                    return None
            return gen

        def emit_a2a(g):
            if mock_collectives:
                return nc.sync.dma_start(attn_g[g].ap()[:], attn_loc[g].ap()[:])
            # f32-bitcast views: the collective path is ~4x slower on
            # 2-byte dtypes, and AllToAll(bypass) only moves bytes.
            return nc.gpsimd.collective_compute(
                "AllToAll", ALU.bypass,
                replica_groups=[list(range(N_CORES))],
                ins=[attn_loc[g].ap().bitcast(F32).opt()],
                outs=[attn_g[g].ap().bitcast(F32).opt()],
            )

        wo_r = wo_d.rearrange("p (ob hc n) -> p ob hc n", ob=8, hc=KC)

        def load_wo_block(ob):
            tiles = []
            for g in range(8):
                wt = wop.tile([128, 4, TB], BF16, name="wo_t", tag="wo")
                nc.sync.dma_start(
                    wt[:], wo_r[:, ob, g * 4:(g + 1) * 4, :])
                tiles.append(wt)
            return tiles

        attn_sb = [None] * 4

        def load_attn_sb(g, cc, half=None):
            # per-hc loads: each is a contiguous 128-row block of attn_g,
            # so every partition reads one contiguous 256B run. `half`
            # splits the 1MB burst across two emission points.
            if half in (None, 0):
                attn_sb[g] = ap3.tile([128, KC, 128], BF16,
                                      name=f"attn_sb{g}")
            t = attn_sb[g]
            hcs = range(KC) if half is None else \
                range(half * KC // 2, (half + 1) * KC // 2)
            for hc in hcs:
                dma = nc.sync.dma_start(
                    t[:, hc, :],
                    attn_g[g].ap()[hc * 128:(hc + 1) * 128, :])
                if cc is not None:
                    add_dep_helper(dma.ins, cc,
                                   reason="attn_sb load after AllToAll")

        wo_g0 = None
        cc_ins = {}
        prev_gen = None
        prev_steps = 0
        for tb in range(NTB):
            b, qb = tb // QBPB, tb % QBPB
            s0 = qb * TB
            resv = 20 if prev_steps > 20 else 0
            per_kc = max(1, -(-(prev_steps - resv) // (2 * KC)))  # ceil
            # ---- projections for tb in two half-passes, interleaved
            # with attention(tb-1); rope for the first half between them.
            # tb0 has no attention to interleave, so it runs a single
            # fused pass (6 MMs per xt tile — half the DMA supply rate),
            # borrowing the idle ps/pa banks for the extra accumulators.
            qps01 = [pp.tile([128, TB], F32, name=f"qps{h}", tag="proj")
                     for h in range(2)]
            kps = pp.tile([128, TB], F32, name="kps", tag="proj")
            if tb == 0:
                qps23 = [ps.tile([128, TB], F32, name=f"qps{h+2}", tag="sps")
                         for h in range(2)]
                vtps = pa.tile([128, TB], F32, name="vtps", tag="aps")
            for kc in range(KC):
                xt_t = pre_xt[kc // 4][:, kc % 4, :]
                for h in range(2):
                    nc.tensor.matmul(
                        qps01[h][:], wq_sb[:, kc, h * 128:(h + 1) * 128],
                        xt_t, start=(kc == 0), stop=(kc == KC - 1),
                        skip_group_check=True)
                nc.tensor.matmul(kps[:], wk_sb[:, kc, :], xt_t,
                                 start=(kc == 0), stop=(kc == KC - 1),
                                 skip_group_check=True)
                if tb == 0:
                    for h in range(2):
                        nc.tensor.matmul(
                            qps23[h][:],
                            wq_sb[:, kc, (h + 2) * 128:(h + 3) * 128],
                            xt_t, start=(kc == 0), stop=(kc == KC - 1),
                            skip_group_check=True)
                    nc.tensor.matmul(vtps[:], wv_sb[:, kc, :], xt_t,
                                     start=(kc == 0), stop=(kc == KC - 1),
                                     skip_group_check=True)
                    if kc % 4 == 3:
                        pre_xt[kc // 4] = xt_dma(1, kc // 4)
                prev_gen = drive(prev_gen, per_kc)

            # rope for q0, q1 and K (drains first-pass PSUM banks)
            qt_tiles = []

            def rope_unit(src_ps, dst_ap):
                """dst = src*cos + rotate_half(src)*sin (PSUM f32 -> bf16)"""
                raw = qrawp.tile([128, TB], BF16, name="qraw", tag="qraw")
                nc.scalar.activation(raw[:], src_ps[:], AF.Copy)
                rotps = ps.tile([128, TB], F32, name="rotps", tag="sps")
                nc.tensor.matmul(rotps[:], rot_sb[:], raw[:],
                                 start=True, stop=True, skip_group_check=True)
                tcos = tmpp.tile([128, TB], F32, name="tcos", tag="tmp")
                nc.vector.tensor_tensor(tcos[:], raw[:],
                                        cos_sb[:, s0:s0 + TB], ALU.mult)
                tsin = tmpp.tile([128, TB], F32, name="tsin", tag="tmp")
                nc.vector.tensor_tensor(tsin[:], rotps[:],
                                        sin_sb[:, s0:s0 + TB], ALU.mult)
                nc.vector.tensor_tensor(dst_ap, tcos[:], tsin[:], ALU.add)

            for h in range(2):
                qt = qtp.tile([128, TB], BF16, name="qt", tag="qt")
                rope_unit(qps01[h], qt[:])
                qt_tiles.append(qt)
                prev_gen = drive(prev_gen, 2)
            rope_unit(kps, kt_sb[b][:, s0:s0 + TB])
            prev_gen = drive(prev_gen, 2)

            # ---- second half-pass: q2, q3, V -----------------------
            if tb > 0:
                qps23 = [pp.tile([128, TB], F32, name=f"qps{h+2}",
                                 tag="proj") for h in range(2)]
                vtps = pp.tile([128, TB], F32, name="vtps", tag="proj")
                for kc in range(KC):
                    xt_t = pre_xt[kc // 4][:, kc % 4, :]
                    for h in range(2):
                        nc.tensor.matmul(
                            qps23[h][:],
                            wq_sb[:, kc, (h + 2) * 128:(h + 3) * 128],
                            xt_t, start=(kc == 0), stop=(kc == KC - 1),
                            skip_group_check=True)
                    nc.tensor.matmul(vtps[:], wv_sb[:, kc, :], xt_t,
                                     start=(kc == 0), stop=(kc == KC - 1),
                                     skip_group_check=True)
                    # one-tb-ahead xt prefetch (after last read of group)
                    if tb < NTB - 1 and kc % 4 == 3:
                        pre_xt[kc // 4] = xt_dma(tb + 1, kc // 4)
                    prev_gen = drive(prev_gen, per_kc)

            for h in range(2):
                qt = qtp.tile([128, TB], BF16, name="qt", tag="qt")
                rope_unit(qps23[h], qt[:])
                qt_tiles.append(qt)
                prev_gen = drive(prev_gen, 2)
            # V: drain V^T then transpose 4x [128,128]
            vtraw = vtrawp.tile([128, TB], BF16, name="vtraw", tag="vtraw")
            nc.scalar.activation(vtraw[:], vtps[:], AF.Copy)
            vtr = pp.tile([128, TB], BF16, name="vtr", tag="proj")
            for t4 in range(4):
                nc.tensor.transpose(vtr[:, t4 * 128:(t4 + 1) * 128],
                                    vtraw[:, t4 * 128:(t4 + 1) * 128],
                                    ident_sb[:])
            nc.vector.tensor_copy(
                v_sb[b].rearrange("p jc d -> p (jc d)")[:, s0:s0 + TB],
                vtr[:])
            prev_gen = drive(prev_gen, 10 ** 9)  # flush any leftovers
            # fire the A2A for the group completed by attention(tb-1);
            # the burst overlaps the next tb's dense projection stream.
            if tb in (2, 4, 6):
                g = tb // 2 - 1
                cc = emit_a2a(g)
                cc_ins[g] = cc.ins
            if tb < NTB - 1:
                prev_gen = emit_attention(tb, ps, pa, qt_tiles)
                prev_steps = HPC * (2 * ((qb + 1) * 4) + 2)
                if tb == 3:
                    load_attn_sb(0, cc_ins[0])
                if tb == 4:
                    wo_g0 = load_wo_block(0)
                if tb == 5:
                    load_attn_sb(1, cc_ins[1])
            else:
                load_attn_sb(2, cc_ins[2])
                tail_qt = qt_tiles

        # ---- free projection-only pools -----------------------------
        pa_cm.__exit__(None, None, None)
        ps_cm.__exit__(None, None, None)
        pp_cm.__exit__(None, None, None)
        for cm in reversed(trans):
            cm.__exit__(None, None, None)
        persistA_cm.__exit__(None, None, None)

        # ---- attention tail (tb=7) with generous psum buffering -----
        ps2_cm = tc.tile_pool(name="ps2", bufs=5, space="PSUM")
        ps2 = ps2_cm.__enter__()
        pa2_cm = tc.tile_pool(name="pa2", bufs=2, space="PSUM")
        pa2 = pa2_cm.__enter__()
        drive(emit_attention(NTB - 1, ps2, pa2, tail_qt, lag=3), 10 ** 9)
        cc3 = emit_a2a(3)
        load_attn_sb(3, cc3.ins)
        pa2_cm.__exit__(None, None, None)
        ps2_cm.__exit__(None, None, None)

        # ---- phase 3: y = attn_rows @ wo ----------------------------
        py_cm = tc.tile_pool(name="py", bufs=4, space="PSUM")
        pyp = py_cm.__enter__()
        for ob in range(8):
            wo_g = wo_g0 if ob == 0 else load_wo_block(ob)
            for tc4 in range(4):
                yps = pyp.tile([128, TB], F32, name="yps", tag="yps")
                for hc in range(KC):
                    nc.tensor.matmul(
                        yps[:], attn_sb[tc4][:, hc, :],
                        wo_g[hc // 4][:, hc % 4, :],
                        start=(hc == 0), stop=(hc == KC - 1),
                        skip_group_check=True)
                y_sb = ysbp.tile([128, TB], F32, name="y_sb", tag="y")
                nc.vector.tensor_copy(y_sb[:], yps[:])
                nc.sync.dma_start(
                    y_d[tc4 * 128:(tc4 + 1) * 128,
                        ob * TB:(ob + 1) * TB], y_sb[:])
        py_cm.__exit__(None, None, None)
        for cm in reversed(tier2):
            cm.__exit__(None, None, None)
        persistB_cm.__exit__(None, None, None)

    nc.compile()
    return nc



_NC_CACHE = None


def _get_nc():
    global _NC_CACHE
    if _NC_CACHE is None:
        _NC_CACHE = build_attn_nc()
    return _NC_CACHE


def _host_reference(x, wq, wk, wv, wo, sincos, start_pos, causal_mask):
    """Numpy fallback (only used if the mask is not causal-tril)."""
    xq = (x @ wq).reshape(B, S, H, HD)
    xk = (x @ wk).reshape(B, S, KH, HD)
    xv = (x @ wv).reshape(B, S, KH, HD)
    sp = min(max(int(start_pos), 0), MS - S)
    sc = sincos[sp:sp + S]
    sin, cos = sc[:, :HD], sc[:, HD:]
    sin = sin[None, :, None, :]
    cos = cos[None, :, None, :]

    def rot(u):
        return np.concatenate([-u[..., HD // 2:], u[..., :HD // 2]], axis=-1)

    xq = xq * cos + rot(xq) * sin
    xk = xk * cos + rot(xk) * sin
    mask = np.broadcast_to(causal_mask[:, sp:sp + S, :MS], (B, S, MS))
    out = np.zeros((B, S, H, HD), dtype=np.float32)
    nrep = H // KH
    for b in range(B):
        for h in range(H):
            q = xq[b, :, h]
            k = xk[b, :, h // nrep]
            v = xv[b, :, h // nrep]
            s = (q @ k.T) * SCALE
            s = np.where(mask[b], s, -np.inf)
            s = s - s.max(axis=-1, keepdims=True)
            p = np.exp(s)
            p /= p.sum(axis=-1, keepdims=True)
            out[b, :, h] = p @ v
    return out.reshape(B, S, H * HD) @ wo


def kernel(x, wq, wk, wv, wo, cache_k, cache_v, sincos, causal_mask,
           start_pos):
    x = np.asarray(x, dtype=np.float32)
    wq = np.asarray(wq, dtype=np.float32)
    wk = np.asarray(wk, dtype=np.float32)
    wv = np.asarray(wv, dtype=np.float32)
    wo = np.asarray(wo, dtype=np.float32)
    sincos = np.asarray(sincos, dtype=np.float32)
    cm = np.asarray(causal_mask)
    sp = min(max(int(start_pos), 0), MS - S)

    tril = np.tril(np.ones((S, MS), dtype=bool))
    if not np.array_equal(cm[0, sp:sp + S, :], tril[:, :MS]):
        return _host_reference(x, wq, wk, wv, wo, sincos, start_pos,
                               cm).astype(np.float32)

    # host prep
    sc = sincos[sp:sp + S]
    sinT = np.ascontiguousarray(sc[:, :HD].T)       # [HD, S]
    cosT = np.ascontiguousarray(sc[:, HD:].T)       # [HD, S]
    xtT = x.reshape(BS, D).T  # [D, BS]
    # pack to (p, tb, kcg, kcs, tok)
    xt = to_bf16(np.ascontiguousarray(
        xtT.reshape(KC // 4, 4, 128, NTB, TB)
           .transpose(2, 3, 0, 1, 4).reshape(128, -1)))
    wqs = wq * np.float32(SCALE)
    wo_b = to_bf16(np.ascontiguousarray(
        wo.reshape(KC, 128, 8, TB).transpose(1, 2, 0, 3).reshape(128, -1)))

    maskd = np.zeros((128, 4, TB), dtype=np.float32)
    j = np.arange(128)[:, None, None]
    r = np.arange(4)[None, :, None]
    q = np.arange(TB)[None, None, :]
    maskd[(r * 128 + j) <= q] = 1.0

    rotm = np.zeros((HD, HD), dtype=np.float32)
    hh = HD // 2
    rotm[np.arange(hh) + hh, np.arange(hh)] = -1.0
    rotm[np.arange(hh), np.arange(hh) + hh] = 1.0

    ident = np.eye(128, dtype=np.float32)
    ones128 = np.ones((128, 128), dtype=np.float32)

    def pack_w(w):
        # [D, cols] -> partition-major [128, KC*cols]
        cols = w.shape[1]
        return to_bf16(np.ascontiguousarray(
            w.reshape(KC, 128, cols).transpose(1, 0, 2).reshape(128, -1)))

    in_maps = []
    for c in range(N_CORES):
        in_maps.append({
            "xt": xt,
            "wq": pack_w(wqs[:, c * QF:(c + 1) * QF]),
            "wk": pack_w(wk[:, c * HD:(c + 1) * HD]),
            "wv": pack_w(wv[:, c * HD:(c + 1) * HD]),
            "wo": wo_b,
            "cosT": to_bf16(cosT), "sinT": to_bf16(sinT),
            "maskd": to_bf16(maskd), "rotm": to_bf16(rotm),
            "ident": to_bf16(ident),
            "ones128": to_bf16(ones128),
        })

    global _LAST_IN_MAPS
    _LAST_IN_MAPS = in_maps
    nc = _get_nc()
    res = run_bass_kernel_spmd(nc, in_maps, list(range(N_CORES)))
    # per-core y rows: 4 slabs of 128 tokens, one per a2a token group g
    # (global tokens g*1024 + c*128 .. +128, b-major flat order).
    y = np.empty((BS, D), dtype=np.float32)
    for c in range(N_CORES):
        yc = res.results[c]["y"]
        for g in range(4):
            y[g * 1024 + c * 128:g * 1024 + (c + 1) * 128] = \
                yc[g * 128:(g + 1) * 128]
    return y.reshape(B, S, D)
